# revision 3
# baseline (speedup 1.0000x reference)
"""Trainium2 kernel for nn_BaselineRelationalIndependentModel:
out = sigmoid(W2d[x, y]) with W2d = W.reshape(2048, 2048), B = 16,777,216.

Sharding: data-parallel — batch split evenly across the 8 NeuronCores; the
16 MiB weight table is replicated (each core reads it from its own HBM).

Device kernel (per core, 2,097,152 lookups laid out [128, 16384]):
  1. flat = 2048*x + y on VectorE (int32 shift/or).
  2. Gather W[flat] via gpsimd indirect DMA: each call consumes one uint32
     element-offset per partition and fetches table[off[p]] into an SBUF
     column — 128 arbitrary-position lookups per call, no index routing
     required anywhere.
  3. sigmoid on ScalarE, stream result back to HBM.
"""

import numpy as np

import concourse.bass as bass
import concourse.bacc as bacc
import concourse.mybir as mybir
import concourse.tile as tile
from concourse.bass_utils import run_bass_kernel_spmd

NOBJ = 2048
TAB = NOBJ * NOBJ          # 4,194,304 table entries
B = 16777216
NCORES = 8
BPC = B // NCORES          # 2,097,152 lookups per core
P = 128
F = BPC // P               # 16384 columns per core
CB = 2048                  # columns per pipeline block


def build_nc(f_total: int = F, cb: int = CB) -> bacc.Bacc:
    nc = bacc.Bacc(None, target_bir_lowering=False)
    xd = nc.dram_tensor("x", [P, f_total], mybir.dt.int32, kind="ExternalInput")
    yd = nc.dram_tensor("y", [P, f_total], mybir.dt.int32, kind="ExternalInput")
    wd = nc.dram_tensor("w", [TAB, 1], mybir.dt.float32, kind="ExternalInput")
    od = nc.dram_tensor("out", [P, f_total], mybir.dt.float32, kind="ExternalOutput")

    nblocks = (f_total + cb - 1) // cb
    with tile.TileContext(nc) as tc:
        with (
            tc.tile_pool(name="io", bufs=3) as io,
            tc.tile_pool(name="mid", bufs=2) as mid,
        ):
            for blk in range(nblocks):
                c0 = blk * cb
                c1 = min(c0 + cb, f_total)
                w = c1 - c0

                xb = io.tile([P, cb], mybir.dt.int32, tag="xb")
                yb = io.tile([P, cb], mybir.dt.int32, tag="yb")
                nc.sync.dma_start(out=xb[:, :w], in_=xd[:, c0:c1])
                nc.sync.dma_start(out=yb[:, :w], in_=yd[:, c0:c1])

                flat = mid.tile([P, cb], mybir.dt.int32, tag="flat")
                nc.vector.tensor_scalar(
                    out=flat[:, :w], in0=xb[:, :w], scalar1=11, scalar2=None,
                    op0=mybir.AluOpType.logical_shift_left,
                )
                nc.vector.tensor_tensor(
                    out=flat[:, :w], in0=flat[:, :w], in1=yb[:, :w],
                    op=mybir.AluOpType.bitwise_or,
                )

                val = mid.tile([P, cb], mybir.dt.float32, tag="val")
                offs = flat[:, :w].bitcast(mybir.dt.uint32)
                for m in range(w):
                    nc.gpsimd.indirect_dma_start(
                        out=val[:, m:m + 1],
                        out_offset=None,
                        in_=wd[:],
                        in_offset=bass.IndirectOffsetOnAxis(ap=offs[:, m:m + 1], axis=0),
                    )

                res = io.tile([P, cb], mybir.dt.float32, tag="res")
                nc.scalar.activation(
                    out=res[:, :w], in_=val[:, :w],
                    func=mybir.ActivationFunctionType.Sigmoid,
                )
                nc.sync.dma_start(out=od[:, c0:c1], in_=res[:, :w])
    nc.compile()
    return nc


# Set by test harnesses to capture an NTFF profile; the graded path leaves
# this False (no tracing dependencies).
TRACE = False
LAST_EXEC_NS = None

_nc_cache: dict[tuple, bacc.Bacc] = {}


def _get_nc(f_total: int = F, cb: int = CB) -> bacc.Bacc:
    key = (f_total, cb)
    if key not in _nc_cache:
        _nc_cache[key] = build_nc(f_total, cb)
    return _nc_cache[key]


def kernel(x: np.ndarray, y: np.ndarray, W: np.ndarray) -> np.ndarray:
    assert x.shape == (B,) and y.shape == (B,)
    x32 = np.ascontiguousarray(np.asarray(x).astype(np.int32, copy=False)).reshape(NCORES, P, F)
    y32 = np.ascontiguousarray(np.asarray(y).astype(np.int32, copy=False)).reshape(NCORES, P, F)
    w = np.ascontiguousarray(np.asarray(W, dtype=np.float32).reshape(TAB, 1))

    nc = _get_nc()
    in_maps = [{"x": x32[c], "y": y32[c], "w": w} for c in range(NCORES)]
    res = run_bass_kernel_spmd(
        nc, in_maps, core_ids=list(range(NCORES)), trace=TRACE
    )
    global LAST_EXEC_NS
    LAST_EXEC_NS = res.exec_time_ns
    out = np.concatenate([res.results[c]["out"].reshape(BPC) for c in range(NCORES)])
    return out[:, None]
